# revision 6
# baseline (speedup 1.0000x reference)
"""DeepFFM Trainium2 kernel (8 NeuronCores, SPMD via bass/Tile).

Math (reference):
  linear      = X @ w1 + b
  S[i,j]      = <nfk[i, f2f[j], :], nfk[j, f2f[i], :]>   (symmetric, param-only)
  interaction = sum_{i<j} S[i,j] X[:,i] X[:,j] = 0.5 * rowsum((X @ Sz) * X)
                where Sz = S with zeroed diagonal (uses symmetry of S)
  deep        = MLP(X) with relu layers
  out         = linear + interaction + deep

Strategy:
  * Host-side: sort features by field (permutation). All float tensors are
    permuted / transposed / cast to bf16 host-side (layout transforms only;
    all FLOPs on device). In sorted order, S splits into per-(field g, field
    f) rank-40 blocks  S[J_g, J_f] = nfkT[f-rows, J_g]^T @ nfkT[g-rows, J_f]
    which are contiguous slices of nfkT = nfk.reshape(F, FIELDS*K).T.
  * S rows are sharded over the 8 cores (5 field groups per core, group list
    padded to 40 groups). Per-core variation lives entirely in DATA (each
    core's in_map carries its own nfkT column/row slices in a fixed padded
    local layout) so the SPMD instruction stream is identical on all cores.
  * Sharded S blocks -> AllGather (bf16, launched as soon as the S shard is
    written — it overlaps the input DMA stream and the deep MLP) -> each
    core rebuilds the full Sz (compact, diagonal zeroed via affine_select).
  * Batch is sharded 512 rows/core for linear+deep+interaction. Activations
    stay transposed (hT = W^T @ hT_prev) so only XT ([F, batch]) is needed.
  * Everything flows in bf16 (PSUM accumulation is fp32): halves all HBM
    traffic and runs every matmul at the full 1 cycle/row PE rate.
  * DMA queueing: TRN2 has exactly two hardware DGE queues (sync, scalar).
    Bulk loads are split across them by need-time; small / late tensors ride
    gpsimd's software DGE so they never block the critical streams.
"""

import numpy as np

import concourse.bass as bass
import concourse.bacc as bacc
import concourse.mybir as mybir
import concourse.tile as tile
from concourse.tile_rust import add_dep_helper
from concourse.bass_utils import run_bass_kernel_spmd

F32 = mybir.dt.float32
F32R = mybir.dt.float32r
BF16 = mybir.dt.bfloat16
DEEP_DT = BF16    # deep-chain matmul dtype
SDT = BF16        # S phase / AllGather dtype
XDT = BF16        # XT dtype

NCORES = 8
B = 4096
BS = B // NCORES          # batch rows per core
F = 1000                  # feature size
FIELDS = 39
NGT = 40                  # padded group count (group 39 empty)
GPC = NGT // NCORES       # groups per core = 5
K = 40                    # ffm embedding dim
D0, D1, D2 = 1024, 512, 256
KT0 = 8                   # k-chunks (125) over F
P_F = 125                 # partition chunk of F


def _build_program(off, counts, pad, bias_const, sim_single_core=False, repeat=None):
    """Trace the (SPMD-identical) bass program. off/counts describe the
    globally-sorted field layout; pad is the per-group local row pitch."""
    lrows = GPC * pad
    nc = bacc.Bacc(None, num_devices=NCORES)

    # All inputs are host-prearranged to their exact SBUF tile layouts so
    # every load is a single contiguous DMA.
    xt_h = nc.dram_tensor("xt", [P_F, KT0, BS], XDT, kind="ExternalInput")
    w0_h = nc.dram_tensor("w0", [P_F, 2, KT0, D1], DEEP_DT, kind="ExternalInput")
    w1_h = nc.dram_tensor("w1w", [128, 8, D1], DEEP_DT, kind="ExternalInput")
    w2_h = nc.dram_tensor("w2w", [128, 4, D2], DEEP_DT, kind="ExternalInput")
    ow_h = nc.dram_tensor("outw", [128, 2, 1], DEEP_DT, kind="ExternalInput")
    wl_h = nc.dram_tensor("w1lin", [P_F, KT0, 1], XDT, kind="ExternalInput")
    b0_h = nc.dram_tensor("b0", [128, 8], F32, kind="ExternalInput")
    b1_h = nc.dram_tensor("b1", [128, 4], F32, kind="ExternalInput")
    b2_h = nc.dram_tensor("b2", [128, 2], F32, kind="ExternalInput")
    nk_h = nc.dram_tensor("nfkt_cols", [K, FIELDS, lrows], SDT, kind="ExternalInput")
    gt_h = nc.dram_tensor("gtiles", [K, GPC, F], SDT, kind="ExternalInput")
    hv_h = nc.dram_tensor("halves", [P_F, 1], XDT, kind="ExternalInput")
    out_h = nc.dram_tensor("out", [1, BS], F32, kind="ExternalOutput")

    # column segments of each field block, split at PSUM bank (512) boundaries
    def col_segs(f):
        c0, c1 = int(off[f]), int(off[f + 1])
        segs = []
        while c0 < c1:
            nxt = min(c1, (c0 // 512 + 1) * 512)
            segs.append((c0, nxt))
            c0 = nxt
        return segs

    # reload map: AG-out padded rows -> compact global rows, split at 125-tiles
    reload_segs = []
    for g in range(FIELDS):
        c, gl = divmod(g, GPC)
        src = c * lrows + gl * pad
        dst = int(off[g])
        n = int(counts[g])
        while n > 0:
            t = dst // P_F
            po = dst % P_F
            take = min(n, P_F - po)
            reload_segs.append((src, t, po, take))
            src += take
            dst += take
            n -= take

    with tile.TileContext(nc) as tc:
        with (
            tc.tile_pool(name="persist", bufs=1) as persist,
            tc.tile_pool(name="sphase", bufs=1) as sphase,
            tc.tile_pool(name="evac", bufs=2) as evac,
            tc.tile_pool(name="work", bufs=2) as work,
            tc.tile_pool(name="psum", bufs=1, space="PSUM") as psum,
            tc.tile_pool(name="dram", bufs=1, space="DRAM") as dram,
        ):
            import contextlib
            rep_ctx = (tc.For_i(0, repeat, 1) if repeat is not None
                       else contextlib.nullcontext())
            with rep_ctx:
                # ---------------- loads ----------------
                # sync HW queue: S-phase columns first, then the agin stores
                # (issued inside the S loop), then w0's 2nd half + w1.
                # scalar HW queue: gtiles, then the deep-critical xt/w0a
                # stream, then all the small tensors.
                # gpsimd carries NOTHING before the collective trigger —
                # software-DGE descriptor generation would delay it.
                nfkt_sb = sphase.tile([K, FIELDS, lrows], SDT)
                nc.sync.dma_start(out=nfkt_sb, in_=nk_h[:])
                gt_sb = sphase.tile([K, GPC, F], SDT)
                nc.scalar.dma_start(out=gt_sb, in_=gt_h[:])
                xt_sb = persist.tile([P_F, KT0, BS], XDT)
                nc.scalar.dma_start(out=xt_sb, in_=xt_h[:])
                w0_sb = persist.tile([P_F, 2, KT0, D1], DEEP_DT)
                nc.scalar.dma_start(out=w0_sb[:, 0, :, :], in_=w0_h[:, 0, :, :])
                b0_sb = persist.tile([128, 8], F32)
                nc.scalar.dma_start(out=b0_sb, in_=b0_h[:])
                b1_sb = persist.tile([128, 4], F32)
                nc.scalar.dma_start(out=b1_sb, in_=b1_h[:])
                b2_sb = persist.tile([128, 2], F32)
                nc.scalar.dma_start(out=b2_sb, in_=b2_h[:])
                w2_sb = persist.tile([128, 4, D2], DEEP_DT)
                nc.scalar.dma_start(out=w2_sb, in_=w2_h[:])
                ow_sb = persist.tile([128, 2, 1], DEEP_DT)
                nc.scalar.dma_start(out=ow_sb, in_=ow_h[:])
                wl_sb = persist.tile([P_F, KT0, 1], XDT)
                nc.scalar.dma_start(out=wl_sb, in_=wl_h[:])
                halves = persist.tile([P_F, 1], XDT)
                nc.scalar.dma_start(out=halves, in_=hv_h[:])

                # PE warm-up: the HAM clock gate needs sustained PE activity
                # to ramp.  Burn the initial DMA-wait window with dummy
                # matmuls on a memset scratch tile.
                warm_sb = work.tile([128, 128], BF16, tag="warm", bufs=1)
                nc.vector.memset(warm_sb, 1.0)
                ps_w = psum.tile([128, 64], F32, tag="ps_o", bufs=1)
                for _ in range(16):
                    nc.tensor.matmul(
                        ps_w, lhsT=warm_sb[:, 0:128], rhs=warm_sb[:, 0:64],
                        start=True, stop=True,
                    )

                agin = dram.tile([lrows, F], SDT)
                agout = dram.tile(
                    [NCORES * lrows, F], SDT,
                    addr_space="Local" if sim_single_core else "Shared",
                )

                # ---------------- S phase: per-group block matmuls ----------------
                for gl in range(GPC):
                    ps_s = psum.tile([pad, F], F32, tag="ps_s", bufs=2)
                    for f in range(FIELDS):
                        for (c0, c1) in col_segs(f):
                            nc.tensor.matmul(
                                ps_s[:, c0:c1],
                                lhsT=nfkt_sb[:, f, gl * pad : (gl + 1) * pad],
                                rhs=gt_sb[:, gl, c0:c1],
                                start=True,
                                stop=True,
                            )
                    srow = evac.tile([pad, F], SDT, tag="srow")
                    nc.vector.tensor_copy(srow, ps_s)
                    nc.sync.dma_start(out=agin[gl * pad : (gl + 1) * pad, :], in_=srow)

                if sim_single_core:
                    # Timeline-sim stand-in for the AllGather (single-core cost
                    # model can't simulate collectives): copy the shard into all 8
                    # rank slots — writes every agout byte (correct deps for the
                    # reload DMAs) and costs ~the real AG wire time.
                    for r in range(NCORES):
                        nc.sync.dma_start(
                            out=agout[r * lrows : (r + 1) * lrows, :], in_=agin[:]
                        )
                else:
                    nc.gpsimd.collective_compute(
                        "AllGather",
                        mybir.AluOpType.bypass,
                        replica_groups=[list(range(NCORES))],
                        ins=[agin[:].opt()],
                        outs=[agout[:].opt()],
                    )

                # w0's 2nd half + w1 ride the sync queue behind the agin
                # stores (sync is otherwise idle from here on).
                nc.sync.dma_start(out=w0_sb[:, 1, :, :], in_=w0_h[:, 1, :, :])
                w1_sb = persist.tile([128, 8, D1], DEEP_DT)
                nc.sync.dma_start(out=w1_sb, in_=w1_h[:])

                # keep the HAM clock warm across the xt/w0 DMA-wait gap
                for _ in range(10):
                    nc.tensor.matmul(
                        ps_w, lhsT=warm_sb[:, 0:128], rhs=warm_sb[:, 0:64],
                        start=True, stop=True,
                    )

                # ---------------- deep MLP (overlaps the collective) -------------
                h0_sb = persist.tile([128, 8, D1], BF16)
                ps_o = psum.tile([1, BS], F32, tag="ps_o", bufs=1)
                for mj in range(8):
                    ps0 = psum.tile([128, BS], F32, tag="ps_mm", bufs=3)
                    for t in range(KT0):
                        nc.tensor.matmul(
                            ps0,
                            lhsT=w0_sb[:, mj // 4, t, (mj % 4) * 128 : (mj % 4 + 1) * 128],
                            rhs=xt_sb[:, t, :],
                            start=(t == 0),
                            stop=(t == KT0 - 1),
                        )
                    nc.scalar.activation(
                        h0_sb[:, mj, :],
                        ps0,
                        mybir.ActivationFunctionType.Relu,
                        bias=b0_sb[:, mj : mj + 1],
                    )
                h1_sb = persist.tile([128, 4, BS], BF16)
                for mj in range(4):
                    ps1 = psum.tile([128, BS], F32, tag="ps_mm", bufs=3)
                    for t in range(8):
                        nc.tensor.matmul(
                            ps1,
                            lhsT=w1_sb[:, t, mj * 128 : (mj + 1) * 128],
                            rhs=h0_sb[:, t, :],
                            start=(t == 0),
                            stop=(t == 7),
                        )
                    nc.scalar.activation(
                        h1_sb[:, mj, :],
                        ps1,
                        mybir.ActivationFunctionType.Relu,
                        bias=b1_sb[:, mj : mj + 1],
                    )
                h2_sb = persist.tile([128, 2, BS], BF16)
                for mj in range(2):
                    ps2 = psum.tile([128, BS], F32, tag="ps_mm", bufs=3)
                    for t in range(4):
                        nc.tensor.matmul(
                            ps2,
                            lhsT=w2_sb[:, t, mj * 128 : (mj + 1) * 128],
                            rhs=h1_sb[:, t, :],
                            start=(t == 0),
                            stop=(t == 3),
                        )
                    nc.scalar.activation(
                        h2_sb[:, mj, :],
                        ps2,
                        mybir.ActivationFunctionType.Relu,
                        bias=b2_sb[:, mj : mj + 1],
                    )
                # ps_o accumulation group: deep head + linear + interaction
                for t in range(2):
                    nc.tensor.matmul(
                        ps_o,
                        lhsT=ow_sb[:, t, :],
                        rhs=h2_sb[:, t, :],
                        start=(t == 0),
                        stop=False,
                    )
                for t in range(KT0):
                    nc.tensor.matmul(
                        ps_o,
                        lhsT=wl_sb[:, t, :],
                        rhs=xt_sb[:, t, :],
                        start=False,
                        stop=False,
                    )

                # keep the HAM clock warm across the AllGather-wait gap so
                # the interaction matmuls run at full rate
                for _ in range(24):
                    nc.tensor.matmul(
                        ps_w, lhsT=warm_sb[:, 0:128], rhs=warm_sb[:, 0:64],
                        start=True, stop=True,
                    )

                # ---------------- rebuild full Sz from the AllGather -------------
                s_sb = persist.tile([P_F, KT0, F], SDT)
                engs = [nc.sync, nc.scalar]
                for i, (src, t, po, n) in enumerate(reload_segs):
                    engs[i % len(engs)].dma_start(
                        out=s_sb[po : po + n, t, :], in_=agout[src : src + n, :]
                    )
                for t in range(KT0):
                    nc.gpsimd.affine_select(
                        out=s_sb[:, t, t * P_F : (t + 1) * P_F],
                        in_=s_sb[:, t, t * P_F : (t + 1) * P_F],
                        compare_op=mybir.AluOpType.not_equal,
                        fill=0.0,
                        base=0,
                        pattern=[[-1, P_F]],
                        channel_multiplier=1,
                    )

                # ---------------- interaction: YT = Sz @ XT, 0.5*colsum(YT*XT) ---
                for mj in range(KT0):
                    ps_y = psum.tile([P_F, BS], F32, tag="ps_mm", bufs=3)
                    for t in range(KT0):
                        nc.tensor.matmul(
                            ps_y,
                            lhsT=s_sb[:, t, mj * P_F : (mj + 1) * P_F],
                            rhs=xt_sb[:, t, :],
                            start=(t == 0),
                            stop=(t == KT0 - 1),
                        )
                    z_sb = work.tile([P_F, BS], XDT, tag="z")
                    nc.vector.tensor_mul(z_sb, ps_y, xt_sb[:, mj, :])
                    nc.tensor.matmul(
                        ps_o,
                        lhsT=halves,
                        rhs=z_sb,
                        start=False,
                        stop=(mj == KT0 - 1),
                    )

                # ---------------- final: add folded scalar bias, store -----------
                out_sb = persist.tile([1, BS], F32)
                nc.vector.tensor_scalar_add(out_sb, ps_o, float(bias_const))
                nc.sync.dma_start(out=out_h[:], in_=out_sb)

    nc.compile()
    return nc


def kernel(X, w1, b, nfk, f2f, deepW0, deepB0, deepW1, deepB1, deepW2, deepB2,
           outW, outB, **_unused):
    import ml_dtypes
    bf16 = ml_dtypes.bfloat16

    X = np.ascontiguousarray(X, dtype=np.float32)
    w1 = np.asarray(w1, dtype=np.float32)
    b = np.asarray(b, dtype=np.float32)
    nfk = np.ascontiguousarray(nfk, dtype=np.float32)
    f2f = np.asarray(f2f)
    deepW0 = np.ascontiguousarray(deepW0, dtype=np.float32)
    deepW1 = np.ascontiguousarray(deepW1, dtype=np.float32)
    deepW2 = np.ascontiguousarray(deepW2, dtype=np.float32)
    outW = np.ascontiguousarray(outW, dtype=np.float32)

    # ---- host-side layout transforms (index/permutation/cast work only) ----
    perm = np.argsort(f2f, kind="stable")
    counts = np.bincount(np.asarray(f2f, dtype=np.int64), minlength=NGT)[:NGT]
    off = np.zeros(NGT + 1, dtype=np.int64)
    off[1:] = np.cumsum(counts)
    pad = int(max(counts.max(), 1))
    lrows = GPC * pad

    XT = np.ascontiguousarray(X[:, perm].T)                     # [F, B]
    w1p = np.ascontiguousarray(w1[perm].reshape(F, 1))
    nfkp = nfk[perm]                                            # [F, FIELDS, K]
    nfkT = np.ascontiguousarray(nfkp.reshape(F, FIELDS * K).T)  # [FIELDS*K, F]
    W0p = np.ascontiguousarray(deepW0[perm])
    bias_const = float(np.float32(b[0]) + np.float32(outB[0]))

    nc = _build_program(off, counts, pad, bias_const)

    def _c(a, dt=bf16):
        return np.ascontiguousarray(a).astype(dt)

    w0_dev = _c(W0p.reshape(KT0, P_F, 2, D1).transpose(1, 2, 0, 3))
    w1_dev = _c(deepW1.reshape(8, 128, D1).transpose(1, 0, 2))
    w2_dev = _c(deepW2.reshape(4, 128, D2).transpose(1, 0, 2))
    ow_dev = _c(outW.reshape(2, 128, 1).transpose(1, 0, 2))
    wl_dev = _c(w1p.reshape(KT0, P_F, 1).transpose(1, 0, 2))
    b0_dev = np.ascontiguousarray(np.asarray(deepB0, np.float32).reshape(8, 128).T)
    b1_dev = np.ascontiguousarray(np.asarray(deepB1, np.float32).reshape(4, 128).T)
    b2_dev = np.ascontiguousarray(np.asarray(deepB2, np.float32).reshape(2, 128).T)
    halves_dev = np.full((P_F, 1), 0.5, dtype=bf16)

    in_maps = []
    for c in range(NCORES):
        nk_cols = np.zeros((FIELDS * K, lrows), dtype=np.float32)
        gtiles = np.zeros((GPC * K, F), dtype=np.float32)
        for gl in range(GPC):
            g = c * GPC + gl
            if g >= FIELDS or counts[g] == 0:
                continue
            nk_cols[:, gl * pad : gl * pad + counts[g]] = (
                nfkT[:, off[g] : off[g + 1]]
            )
            gtiles[gl * K : (gl + 1) * K, :] = nfkT[g * K : (g + 1) * K, :]
        in_maps.append({
            "xt": _c(XT[:, c * BS : (c + 1) * BS].reshape(KT0, P_F, BS).transpose(1, 0, 2)),
            "w0": w0_dev,
            "w1w": w1_dev,
            "w2w": w2_dev,
            "outw": ow_dev,
            "w1lin": wl_dev,
            "b0": b0_dev, "b1": b1_dev, "b2": b2_dev,
            "nfkt_cols": _c(nk_cols.reshape(FIELDS, K, lrows).transpose(1, 0, 2)),
            "gtiles": _c(gtiles.reshape(GPC, K, F).transpose(1, 0, 2)),
            "halves": halves_dev,
        })

    res = run_bass_kernel_spmd(nc, in_maps, core_ids=list(range(NCORES)))
    global LAST_RESULT
    LAST_RESULT = res
    out = np.concatenate([r["out"].reshape(-1) for r in res.results])
    return out.astype(np.float32)


LAST_RESULT = None


if __name__ == "__main__":
    import importlib.util as _iu

    spec = _iu.spec_from_file_location("ref", "/root/problem/reference.py")
    ref = _iu.module_from_spec(spec)
    spec.loader.exec_module(ref)
    inp = {k: np.asarray(v) for k, v in ref.setup_inputs().items()}
    got = kernel(**inp)
    print("kernel out:", got[:8])


# revision 10
# speedup vs baseline: 1.0291x; 1.0291x over previous
"""DeepFFM Trainium2 kernel (8 NeuronCores, SPMD via bass/Tile).

Math (reference):
  linear      = X @ w1 + b
  S[i,j]      = <nfk[i, f2f[j], :], nfk[j, f2f[i], :]>   (symmetric, param-only)
  interaction = sum_{i<j} S[i,j] X[:,i] X[:,j] = 0.5 * rowsum((X @ Sz) * X)
                where Sz = S with zeroed diagonal (uses symmetry of S)
  deep        = MLP(X) with relu layers
  out         = linear + interaction + deep

Strategy:
  * Host-side: sort features by field (permutation). All float tensors are
    permuted / transposed / cast to bf16 host-side (layout transforms only;
    all FLOPs on device). In sorted order, S splits into per-(field g, field
    f) rank-40 blocks  S[J_g, J_f] = nfkT[f-rows, J_g]^T @ nfkT[g-rows, J_f]
    which are contiguous slices of nfkT = nfk.reshape(F, FIELDS*K).T.
  * S rows are sharded over the 8 cores (5 field groups per core, group list
    padded to 40 groups). Per-core variation lives entirely in DATA (each
    core's in_map carries its own nfkT column/row slices in a fixed padded
    local layout) so the SPMD instruction stream is identical on all cores.
  * Sharded S blocks -> AllGather (bf16, launched as soon as the S shard is
    written — it overlaps the input DMA stream and the deep MLP) -> each
    core rebuilds the full Sz (compact, diagonal zeroed via affine_select).
  * Batch is sharded 512 rows/core for linear+deep+interaction. Activations
    stay transposed (hT = W^T @ hT_prev) so only XT ([F, batch]) is needed.
  * Everything flows in bf16 (PSUM accumulation is fp32): halves all HBM
    traffic and runs every matmul at the full 1 cycle/row PE rate.
  * DMA queueing: TRN2 has exactly two hardware DGE queues (sync, scalar).
    Bulk loads are split across them by need-time; small / late tensors ride
    gpsimd's software DGE so they never block the critical streams.
"""

import numpy as np

import concourse.bass as bass
import concourse.bacc as bacc
import concourse.mybir as mybir
import concourse.tile as tile
from concourse.tile_rust import add_dep_helper
from concourse.bass_utils import run_bass_kernel_spmd

F32 = mybir.dt.float32
F32R = mybir.dt.float32r
BF16 = mybir.dt.bfloat16
DEEP_DT = BF16    # deep-chain matmul dtype
SDT = BF16        # S phase / AllGather dtype
XDT = BF16        # XT dtype

NCORES = 8
B = 4096
BS = B // NCORES          # batch rows per core
F = 1000                  # feature size
FIELDS = 39
NGT = 40                  # padded group count (group 39 empty)
GPC = NGT // NCORES       # groups per core = 5
K = 40                    # ffm embedding dim
D0, D1, D2 = 1024, 512, 256
KT0 = 8                   # k-chunks (125) over F
P_F = 125                 # partition chunk of F


def _build_program(off, counts, pad, bias_const, sim_single_core=False, repeat=None):
    """Trace the (SPMD-identical) bass program. off/counts describe the
    globally-sorted field layout; pad is the per-group local row pitch."""
    lrows = GPC * pad
    nc = bacc.Bacc(None, num_devices=NCORES)

    # All inputs are host-prearranged to their exact SBUF tile layouts so
    # every load is a single contiguous DMA.
    xt_h = nc.dram_tensor("xt", [P_F, KT0, BS], XDT, kind="ExternalInput")
    w0_h = nc.dram_tensor("w0", [P_F, 2, KT0, D1], DEEP_DT, kind="ExternalInput")
    w1_h = nc.dram_tensor("w1w", [128, 8, D1], DEEP_DT, kind="ExternalInput")
    w2_h = nc.dram_tensor("w2w", [128, 4, D2], DEEP_DT, kind="ExternalInput")
    ow_h = nc.dram_tensor("outw", [128, 2, 1], DEEP_DT, kind="ExternalInput")
    wl_h = nc.dram_tensor("w1lin", [P_F, KT0, 1], XDT, kind="ExternalInput")
    b0_h = nc.dram_tensor("b0", [128, 8], F32, kind="ExternalInput")
    b1_h = nc.dram_tensor("b1", [128, 4], F32, kind="ExternalInput")
    b2_h = nc.dram_tensor("b2", [128, 2], F32, kind="ExternalInput")
    nk_h = nc.dram_tensor("nfkt_cols", [K, FIELDS, lrows], SDT, kind="ExternalInput")
    gt_h = nc.dram_tensor("gtiles", [K, GPC, F], SDT, kind="ExternalInput")
    hv_h = nc.dram_tensor("halves", [P_F, 1], XDT, kind="ExternalInput")
    out_h = nc.dram_tensor("out", [1, BS], F32, kind="ExternalOutput")

    # column segments of each field block, split at PSUM bank (512) boundaries
    def col_segs(f):
        c0, c1 = int(off[f]), int(off[f + 1])
        segs = []
        while c0 < c1:
            nxt = min(c1, (c0 // 512 + 1) * 512)
            segs.append((c0, nxt))
            c0 = nxt
        return segs

    # reload map: AG-out padded rows -> compact global rows, split at 125-tiles
    reload_segs = []
    for g in range(FIELDS):
        c, gl = divmod(g, GPC)
        src = c * lrows + gl * pad
        dst = int(off[g])
        n = int(counts[g])
        while n > 0:
            t = dst // P_F
            po = dst % P_F
            take = min(n, P_F - po)
            reload_segs.append((src, t, po, take))
            src += take
            dst += take
            n -= take

    with tile.TileContext(nc) as tc:
        with (
            tc.tile_pool(name="persist", bufs=1) as persist,
            tc.tile_pool(name="sphase", bufs=1) as sphase,
            tc.tile_pool(name="evac", bufs=2) as evac,
            tc.tile_pool(name="work", bufs=2) as work,
            tc.tile_pool(name="psum", bufs=1, space="PSUM") as psum,
            tc.tile_pool(name="dram", bufs=1, space="DRAM") as dram,
        ):
            import contextlib
            rep_ctx = (tc.For_i(0, repeat, 1) if repeat is not None
                       else contextlib.nullcontext())
            with rep_ctx:
                # ---------------- loads ----------------
                # sync HW queue: S-phase columns first, then the agin stores
                # (issued inside the S loop), then w0's 2nd half + w1.
                # scalar HW queue: gtiles, then the deep-critical xt/w0a
                # stream, then all the small tensors.
                # gpsimd carries NOTHING before the collective trigger —
                # software-DGE descriptor generation would delay it.
                nfkt_sb = sphase.tile([K, FIELDS, lrows], SDT)
                nc.sync.dma_start(out=nfkt_sb, in_=nk_h[:])
                gt_sb = sphase.tile([K, GPC, F], SDT)
                nc.scalar.dma_start(out=gt_sb, in_=gt_h[:])
                xt_sb = persist.tile([P_F, KT0, BS], XDT)
                nc.scalar.dma_start(out=xt_sb, in_=xt_h[:])
                w0_sb = persist.tile([P_F, 2, KT0, D1], DEEP_DT)
                nc.scalar.dma_start(out=w0_sb[:, 0, :, :], in_=w0_h[:, 0, :, :])
                b0_sb = persist.tile([128, 8], F32)
                nc.scalar.dma_start(out=b0_sb, in_=b0_h[:])
                b1_sb = persist.tile([128, 4], F32)
                nc.scalar.dma_start(out=b1_sb, in_=b1_h[:])
                b2_sb = persist.tile([128, 2], F32)
                nc.scalar.dma_start(out=b2_sb, in_=b2_h[:])
                w2_sb = persist.tile([128, 4, D2], DEEP_DT)
                nc.scalar.dma_start(out=w2_sb, in_=w2_h[:])
                ow_sb = persist.tile([128, 2, 1], DEEP_DT)
                nc.scalar.dma_start(out=ow_sb, in_=ow_h[:])
                wl_sb = persist.tile([P_F, KT0, 1], XDT)
                nc.scalar.dma_start(out=wl_sb, in_=wl_h[:])
                halves = persist.tile([P_F, 1], XDT)
                nc.scalar.dma_start(out=halves, in_=hv_h[:])
                w1_sb = persist.tile([128, 8, D1], DEEP_DT)
                nc.scalar.dma_start(out=w1_sb, in_=w1_h[:])

                # PE warm-up: the HAM clock gate needs sustained PE activity
                # to ramp.  Burn the initial DMA-wait window with dummy
                # matmuls on a memset scratch tile.
                warm_sb = work.tile([128, 128], BF16, tag="warm", bufs=1)
                nc.vector.memset(warm_sb, 1.0)
                ps_w = psum.tile([128, 64], F32, tag="ps_o", bufs=1)
                for _ in range(16):
                    nc.tensor.matmul(
                        ps_w, lhsT=warm_sb[:, 0:128], rhs=warm_sb[:, 0:64],
                        start=True, stop=True,
                    )

                agin = dram.tile([lrows, F], SDT)
                agout = dram.tile(
                    [NCORES * lrows, F], SDT,
                    addr_space="Local" if sim_single_core else "Shared",
                )

                # ---------------- S phase: per-group block matmuls ----------------
                for gl in range(GPC):
                    ps_s = psum.tile([pad, F], F32, tag="ps_s", bufs=2)
                    for f in range(FIELDS):
                        for (c0, c1) in col_segs(f):
                            nc.tensor.matmul(
                                ps_s[:, c0:c1],
                                lhsT=nfkt_sb[:, f, gl * pad : (gl + 1) * pad],
                                rhs=gt_sb[:, gl, c0:c1],
                                start=True,
                                stop=True,
                            )
                    srow = evac.tile([pad, F], SDT, tag="srow")
                    nc.vector.tensor_copy(srow, ps_s)
                    agin_dma = nc.sync.dma_start(
                        out=agin[gl * pad : (gl + 1) * pad, :], in_=srow
                    )

                if sim_single_core:
                    # Timeline-sim stand-in for the AllGather (single-core cost
                    # model can't simulate collectives): copy the shard into all 8
                    # rank slots — writes every agout byte (correct deps for the
                    # reload DMAs) and costs ~the real AG wire time.
                    for r in range(NCORES):
                        nc.sync.dma_start(
                            out=agout[r * lrows : (r + 1) * lrows, :], in_=agin[:]
                        )
                else:
                    nc.gpsimd.collective_compute(
                        "AllGather",
                        mybir.AluOpType.bypass,
                        replica_groups=[list(range(NCORES))],
                        ins=[agin[:].opt()],
                        outs=[agout[:].opt()],
                    )

                # w0's 2nd half rides the sync queue, explicitly ordered
                # BEHIND the last agin store so the tile scheduler cannot
                # hoist it ahead and delay the collective's input.
                w0b_dma = nc.sync.dma_start(out=w0_sb[:, 1, :, :], in_=w0_h[:, 1, :, :])
                add_dep_helper(w0b_dma.ins, agin_dma.ins, sync=True,
                               reason="w0b after agin stores")

                # keep the HAM clock warm across the xt/w0 DMA-wait gap
                for _ in range(10):
                    nc.tensor.matmul(
                        ps_w, lhsT=warm_sb[:, 0:128], rhs=warm_sb[:, 0:64],
                        start=True, stop=True,
                    )

                # ---------------- deep MLP (overlaps the collective) -------------
                h0_sb = persist.tile([128, 8, D1], BF16)
                ps_o = psum.tile([1, BS], F32, tag="ps_o", bufs=1)
                for mj in range(8):
                    ps0 = psum.tile([128, BS], F32, tag="ps_mm", bufs=3)
                    for t in range(KT0):
                        nc.tensor.matmul(
                            ps0,
                            lhsT=w0_sb[:, mj // 4, t, (mj % 4) * 128 : (mj % 4 + 1) * 128],
                            rhs=xt_sb[:, t, :],
                            start=(t == 0),
                            stop=(t == KT0 - 1),
                        )
                    nc.scalar.activation(
                        h0_sb[:, mj, :],
                        ps0,
                        mybir.ActivationFunctionType.Relu,
                        bias=b0_sb[:, mj : mj + 1],
                    )
                h1_sb = persist.tile([128, 4, BS], BF16)
                for mj in range(4):
                    ps1 = psum.tile([128, BS], F32, tag="ps_mm", bufs=3)
                    for t in range(8):
                        nc.tensor.matmul(
                            ps1,
                            lhsT=w1_sb[:, t, mj * 128 : (mj + 1) * 128],
                            rhs=h0_sb[:, t, :],
                            start=(t == 0),
                            stop=(t == 7),
                        )
                    nc.scalar.activation(
                        h1_sb[:, mj, :],
                        ps1,
                        mybir.ActivationFunctionType.Relu,
                        bias=b1_sb[:, mj : mj + 1],
                    )
                h2_sb = persist.tile([128, 2, BS], BF16)
                for mj in range(2):
                    ps2 = psum.tile([128, BS], F32, tag="ps_mm", bufs=3)
                    for t in range(4):
                        nc.tensor.matmul(
                            ps2,
                            lhsT=w2_sb[:, t, mj * 128 : (mj + 1) * 128],
                            rhs=h1_sb[:, t, :],
                            start=(t == 0),
                            stop=(t == 3),
                        )
                    nc.scalar.activation(
                        h2_sb[:, mj, :],
                        ps2,
                        mybir.ActivationFunctionType.Relu,
                        bias=b2_sb[:, mj : mj + 1],
                    )
                # ps_o accumulation group: deep head + linear + interaction
                for t in range(2):
                    nc.tensor.matmul(
                        ps_o,
                        lhsT=ow_sb[:, t, :],
                        rhs=h2_sb[:, t, :],
                        start=(t == 0),
                        stop=False,
                    )
                for t in range(KT0):
                    nc.tensor.matmul(
                        ps_o,
                        lhsT=wl_sb[:, t, :],
                        rhs=xt_sb[:, t, :],
                        start=False,
                        stop=False,
                    )

                # keep the HAM clock warm across the AllGather-wait gap so
                # the interaction matmuls run at full rate
                for _ in range(24):
                    nc.tensor.matmul(
                        ps_w, lhsT=warm_sb[:, 0:128], rhs=warm_sb[:, 0:64],
                        start=True, stop=True,
                    )

                # ---------------- rebuild full Sz from the AllGather -------------
                s_sb = persist.tile([P_F, KT0, F], SDT)
                engs = [nc.sync, nc.scalar]
                for i, (src, t, po, n) in enumerate(reload_segs):
                    engs[i % len(engs)].dma_start(
                        out=s_sb[po : po + n, t, :], in_=agout[src : src + n, :],
                        single_packet=True,
                    )
                for t in range(KT0):
                    nc.gpsimd.affine_select(
                        out=s_sb[:, t, t * P_F : (t + 1) * P_F],
                        in_=s_sb[:, t, t * P_F : (t + 1) * P_F],
                        compare_op=mybir.AluOpType.not_equal,
                        fill=0.0,
                        base=0,
                        pattern=[[-1, P_F]],
                        channel_multiplier=1,
                    )

                # ---------------- interaction: YT = Sz @ XT, 0.5*colsum(YT*XT) ---
                for mj in range(KT0):
                    ps_y = psum.tile([P_F, BS], F32, tag="ps_mm", bufs=3)
                    for t in range(KT0):
                        nc.tensor.matmul(
                            ps_y,
                            lhsT=s_sb[:, t, mj * P_F : (mj + 1) * P_F],
                            rhs=xt_sb[:, t, :],
                            start=(t == 0),
                            stop=(t == KT0 - 1),
                        )
                    z_sb = work.tile([P_F, BS], XDT, tag="z")
                    nc.vector.tensor_mul(z_sb, ps_y, xt_sb[:, mj, :])
                    nc.tensor.matmul(
                        ps_o,
                        lhsT=halves,
                        rhs=z_sb,
                        start=False,
                        stop=(mj == KT0 - 1),
                    )

                # ---------------- final: add folded scalar bias, store -----------
                out_sb = persist.tile([1, BS], F32)
                nc.vector.tensor_scalar_add(out_sb, ps_o, float(bias_const))
                nc.sync.dma_start(out=out_h[:], in_=out_sb)

    nc.compile()
    return nc


def kernel(X, w1, b, nfk, f2f, deepW0, deepB0, deepW1, deepB1, deepW2, deepB2,
           outW, outB, **_unused):
    import ml_dtypes
    bf16 = ml_dtypes.bfloat16

    X = np.ascontiguousarray(X, dtype=np.float32)
    w1 = np.asarray(w1, dtype=np.float32)
    b = np.asarray(b, dtype=np.float32)
    nfk = np.ascontiguousarray(nfk, dtype=np.float32)
    f2f = np.asarray(f2f)
    deepW0 = np.ascontiguousarray(deepW0, dtype=np.float32)
    deepW1 = np.ascontiguousarray(deepW1, dtype=np.float32)
    deepW2 = np.ascontiguousarray(deepW2, dtype=np.float32)
    outW = np.ascontiguousarray(outW, dtype=np.float32)

    # ---- host-side layout transforms (index/permutation/cast work only) ----
    perm = np.argsort(f2f, kind="stable")
    counts = np.bincount(np.asarray(f2f, dtype=np.int64), minlength=NGT)[:NGT]
    off = np.zeros(NGT + 1, dtype=np.int64)
    off[1:] = np.cumsum(counts)
    pad = int(max(counts.max(), 1))
    lrows = GPC * pad

    XT = np.ascontiguousarray(X[:, perm].T)                     # [F, B]
    w1p = np.ascontiguousarray(w1[perm].reshape(F, 1))
    nfkp = nfk[perm]                                            # [F, FIELDS, K]
    nfkT = np.ascontiguousarray(nfkp.reshape(F, FIELDS * K).T)  # [FIELDS*K, F]
    W0p = np.ascontiguousarray(deepW0[perm])
    bias_const = float(np.float32(b[0]) + np.float32(outB[0]))

    nc = _build_program(off, counts, pad, bias_const)

    def _c(a, dt=bf16):
        return np.ascontiguousarray(a).astype(dt)

    w0_dev = _c(W0p.reshape(KT0, P_F, 2, D1).transpose(1, 2, 0, 3))
    w1_dev = _c(deepW1.reshape(8, 128, D1).transpose(1, 0, 2))
    w2_dev = _c(deepW2.reshape(4, 128, D2).transpose(1, 0, 2))
    ow_dev = _c(outW.reshape(2, 128, 1).transpose(1, 0, 2))
    wl_dev = _c(w1p.reshape(KT0, P_F, 1).transpose(1, 0, 2))
    b0_dev = np.ascontiguousarray(np.asarray(deepB0, np.float32).reshape(8, 128).T)
    b1_dev = np.ascontiguousarray(np.asarray(deepB1, np.float32).reshape(4, 128).T)
    b2_dev = np.ascontiguousarray(np.asarray(deepB2, np.float32).reshape(2, 128).T)
    halves_dev = np.full((P_F, 1), 0.5, dtype=bf16)

    in_maps = []
    for c in range(NCORES):
        nk_cols = np.zeros((FIELDS * K, lrows), dtype=np.float32)
        gtiles = np.zeros((GPC * K, F), dtype=np.float32)
        for gl in range(GPC):
            g = c * GPC + gl
            if g >= FIELDS or counts[g] == 0:
                continue
            nk_cols[:, gl * pad : gl * pad + counts[g]] = (
                nfkT[:, off[g] : off[g + 1]]
            )
            gtiles[gl * K : (gl + 1) * K, :] = nfkT[g * K : (g + 1) * K, :]
        in_maps.append({
            "xt": _c(XT[:, c * BS : (c + 1) * BS].reshape(KT0, P_F, BS).transpose(1, 0, 2)),
            "w0": w0_dev,
            "w1w": w1_dev,
            "w2w": w2_dev,
            "outw": ow_dev,
            "w1lin": wl_dev,
            "b0": b0_dev, "b1": b1_dev, "b2": b2_dev,
            "nfkt_cols": _c(nk_cols.reshape(FIELDS, K, lrows).transpose(1, 0, 2)),
            "gtiles": _c(gtiles.reshape(GPC, K, F).transpose(1, 0, 2)),
            "halves": halves_dev,
        })

    res = run_bass_kernel_spmd(nc, in_maps, core_ids=list(range(NCORES)))
    global LAST_RESULT
    LAST_RESULT = res
    out = np.concatenate([r["out"].reshape(-1) for r in res.results])
    return out.astype(np.float32)


LAST_RESULT = None


if __name__ == "__main__":
    import importlib.util as _iu

    spec = _iu.spec_from_file_location("ref", "/root/problem/reference.py")
    ref = _iu.module_from_spec(spec)
    spec.loader.exec_module(ref)
    inp = {k: np.asarray(v) for k, v in ref.setup_inputs().items()}
    got = kernel(**inp)
    print("kernel out:", got[:8])


# revision 11
# speedup vs baseline: 1.0605x; 1.0305x over previous
"""DeepFFM Trainium2 kernel (8 NeuronCores, SPMD via bass/Tile).

Math (reference):
  linear      = X @ w1 + b
  S[i,j]      = <nfk[i, f2f[j], :], nfk[j, f2f[i], :]>   (symmetric, param-only)
  interaction = sum_{i<j} S[i,j] X[:,i] X[:,j] = 0.5 * rowsum((X @ Sz) * X)
                where Sz = S with zeroed diagonal (uses symmetry of S)
  deep        = MLP(X) with relu layers
  out         = linear + interaction + deep

Strategy:
  * Host-side: sort features by field (permutation). All float tensors are
    permuted / transposed / cast to bf16 host-side (layout transforms only;
    all FLOPs on device). In sorted order, S splits into per-(field g, field
    f) rank-40 blocks  S[J_g, J_f] = nfkT[f-rows, J_g]^T @ nfkT[g-rows, J_f]
    which are contiguous slices of nfkT = nfk.reshape(F, FIELDS*K).T.
  * S rows are sharded over the 8 cores (5 field groups per core, group list
    padded to 40 groups). Per-core variation lives entirely in DATA (each
    core's in_map carries its own nfkT column/row slices in a fixed padded
    local layout) so the SPMD instruction stream is identical on all cores.
  * Sharded S blocks -> AllGather (bf16, launched as soon as the S shard is
    written — it overlaps the input DMA stream and the deep MLP) -> each
    core rebuilds the full Sz (compact, diagonal zeroed via affine_select).
  * Batch is sharded 512 rows/core for linear+deep+interaction. Activations
    stay transposed (hT = W^T @ hT_prev) so only XT ([F, batch]) is needed.
  * Everything flows in bf16 (PSUM accumulation is fp32): halves all HBM
    traffic and runs every matmul at the full 1 cycle/row PE rate.
  * DMA queueing: TRN2 has exactly two hardware DGE queues (sync, scalar).
    Bulk loads are split across them by need-time; small / late tensors ride
    gpsimd's software DGE so they never block the critical streams.
"""

import numpy as np

import concourse.bass as bass
import concourse.bacc as bacc
import concourse.mybir as mybir
import concourse.tile as tile
from concourse.tile_rust import add_dep_helper
from concourse.bass_utils import run_bass_kernel_spmd

F32 = mybir.dt.float32
F32R = mybir.dt.float32r
BF16 = mybir.dt.bfloat16
DEEP_DT = BF16    # deep-chain matmul dtype
SDT = BF16        # S phase / AllGather dtype
XDT = BF16        # XT dtype

NCORES = 8
B = 4096
BS = B // NCORES          # batch rows per core
F = 1000                  # feature size
FIELDS = 39
NGT = 40                  # padded group count (group 39 empty)
GPC = NGT // NCORES       # groups per core = 5
K = 40                    # ffm embedding dim
D0, D1, D2 = 1024, 512, 256
KT0 = 8                   # k-chunks (125) over F
P_F = 125                 # partition chunk of F


def _build_program(off, counts, pad, bias_const, sim_single_core=False, repeat=None):
    """Trace the (SPMD-identical) bass program. off/counts describe the
    globally-sorted field layout; pad is the per-group local row pitch."""
    lrows = GPC * pad
    nc = bacc.Bacc(None, num_devices=NCORES)

    # All inputs are host-prearranged to their exact SBUF tile layouts so
    # every load is a single contiguous DMA.
    xt_h = nc.dram_tensor("xt", [P_F, KT0, BS], XDT, kind="ExternalInput")
    w0_h = nc.dram_tensor("w0", [P_F, 2, KT0, D1], DEEP_DT, kind="ExternalInput")
    w1_h = nc.dram_tensor("w1w", [128, 8, D1], DEEP_DT, kind="ExternalInput")
    w2_h = nc.dram_tensor("w2w", [128, 4, D2], DEEP_DT, kind="ExternalInput")
    ow_h = nc.dram_tensor("outw", [128, 2, 1], DEEP_DT, kind="ExternalInput")
    wl_h = nc.dram_tensor("w1lin", [P_F, KT0, 1], XDT, kind="ExternalInput")
    b0_h = nc.dram_tensor("b0", [128, 8], F32, kind="ExternalInput")
    b1_h = nc.dram_tensor("b1", [128, 4], F32, kind="ExternalInput")
    b2_h = nc.dram_tensor("b2", [128, 2], F32, kind="ExternalInput")
    nk_h = nc.dram_tensor("nfkt_cols", [K, FIELDS, lrows], SDT, kind="ExternalInput")
    gt_h = nc.dram_tensor("gtiles", [K, GPC, F], SDT, kind="ExternalInput")
    hv_h = nc.dram_tensor("halves", [P_F, 1], XDT, kind="ExternalInput")
    out_h = nc.dram_tensor("out", [1, BS], F32, kind="ExternalOutput")

    # column segments of each field block, split at PSUM bank (512) boundaries
    def col_segs(f):
        c0, c1 = int(off[f]), int(off[f + 1])
        segs = []
        while c0 < c1:
            nxt = min(c1, (c0 // 512 + 1) * 512)
            segs.append((c0, nxt))
            c0 = nxt
        return segs

    # reload map: AG-out padded rows -> compact global rows, split at 125-tiles.
    # Group g lives on core g%8, local slot g//8 (stride-8 so AG1 = groups
    # 0-23 = the first ~5 compact row chunks, AG2 = groups 24-39).
    reload_segs = []
    for g in range(FIELDS):
        c, gl = g % NCORES, g // NCORES
        if gl < 3:
            which, src = 0, (c * 3 + gl) * pad
        else:
            which, src = 1, (c * 2 + (gl - 3)) * pad
        dst = int(off[g])
        n = int(counts[g])
        while n > 0:
            t = dst // P_F
            po = dst % P_F
            take = min(n, P_F - po)
            reload_segs.append((which, src, t, po, take))
            src += take
            dst += take
            n -= take

    with tile.TileContext(nc) as tc:
        with (
            tc.tile_pool(name="persist", bufs=1) as persist,
            tc.tile_pool(name="sphase", bufs=1) as sphase,
            tc.tile_pool(name="evac", bufs=2) as evac,
            tc.tile_pool(name="work", bufs=2) as work,
            tc.tile_pool(name="psum", bufs=1, space="PSUM") as psum,
            tc.tile_pool(name="dram", bufs=1, space="DRAM") as dram,
        ):
            import contextlib
            rep_ctx = (tc.For_i(0, repeat, 1) if repeat is not None
                       else contextlib.nullcontext())
            with rep_ctx:
                # ---------------- loads ----------------
                # sync HW queue: S-phase columns first, then the agin stores
                # (issued inside the S loop), then w0's 2nd half + w1.
                # scalar HW queue: gtiles, then the deep-critical xt/w0a
                # stream, then all the small tensors.
                # gpsimd carries NOTHING before the collective trigger —
                # software-DGE descriptor generation would delay it.
                nfkt_sb = sphase.tile([K, FIELDS, lrows], SDT)
                nc.sync.dma_start(out=nfkt_sb, in_=nk_h[:])
                gt_sb = sphase.tile([K, GPC, F], SDT)
                nc.scalar.dma_start(out=gt_sb, in_=gt_h[:])
                xt_sb = persist.tile([P_F, KT0, BS], XDT)
                nc.scalar.dma_start(out=xt_sb, in_=xt_h[:])
                w0_sb = persist.tile([P_F, 2, KT0, D1], DEEP_DT)
                nc.scalar.dma_start(out=w0_sb[:, 0, :, :], in_=w0_h[:, 0, :, :])
                b0_sb = persist.tile([128, 8], F32)
                nc.scalar.dma_start(out=b0_sb, in_=b0_h[:])
                b1_sb = persist.tile([128, 4], F32)
                nc.scalar.dma_start(out=b1_sb, in_=b1_h[:])
                b2_sb = persist.tile([128, 2], F32)
                nc.scalar.dma_start(out=b2_sb, in_=b2_h[:])
                w2_sb = persist.tile([128, 4, D2], DEEP_DT)
                nc.scalar.dma_start(out=w2_sb, in_=w2_h[:])
                ow_sb = persist.tile([128, 2, 1], DEEP_DT)
                nc.scalar.dma_start(out=ow_sb, in_=ow_h[:])
                wl_sb = persist.tile([P_F, KT0, 1], XDT)
                nc.scalar.dma_start(out=wl_sb, in_=wl_h[:])
                halves = persist.tile([P_F, 1], XDT)
                nc.scalar.dma_start(out=halves, in_=hv_h[:])
                w1_sb = persist.tile([128, 8, D1], DEEP_DT)
                nc.scalar.dma_start(out=w1_sb, in_=w1_h[:])

                # PE warm-up: the HAM clock gate needs sustained PE activity
                # to ramp.  Burn the initial DMA-wait window with dummy
                # matmuls on a memset scratch tile.
                warm_sb = work.tile([128, 128], BF16, tag="warm", bufs=1)
                nc.vector.memset(warm_sb, 1.0)
                ps_w = psum.tile([128, 64], F32, tag="ps_o", bufs=1)
                for _ in range(16):
                    nc.tensor.matmul(
                        ps_w, lhsT=warm_sb[:, 0:128], rhs=warm_sb[:, 0:64],
                        start=True, stop=True,
                    )

                agin = dram.tile([lrows, F], SDT)
                agout = dram.tile(
                    [NCORES * lrows, F], SDT,
                    addr_space="Local" if sim_single_core else "Shared",
                )

                # ---------------- S phase: per-group block matmuls ----------------
                for gl in range(GPC):
                    ps_s = psum.tile([pad, F], F32, tag="ps_s", bufs=2)
                    for f in range(FIELDS):
                        for (c0, c1) in col_segs(f):
                            nc.tensor.matmul(
                                ps_s[:, c0:c1],
                                lhsT=nfkt_sb[:, f, gl * pad : (gl + 1) * pad],
                                rhs=gt_sb[:, gl, c0:c1],
                                start=True,
                                stop=True,
                            )
                    srow = evac.tile([pad, F], SDT, tag="srow")
                    nc.vector.tensor_copy(srow, ps_s)
                    agin_dma = nc.sync.dma_start(
                        out=agin[gl * pad : (gl + 1) * pad, :], in_=srow
                    )

                if sim_single_core:
                    # Timeline-sim stand-in for the AllGather (single-core cost
                    # model can't simulate collectives): copy the shard into all 8
                    # rank slots — writes every agout byte (correct deps for the
                    # reload DMAs) and costs ~the real AG wire time.
                    for r in range(NCORES):
                        nc.sync.dma_start(
                            out=agout[r * lrows : (r + 1) * lrows, :], in_=agin[:]
                        )
                else:
                    nc.gpsimd.collective_compute(
                        "AllGather",
                        mybir.AluOpType.bypass,
                        replica_groups=[list(range(NCORES))],
                        ins=[agin[:].opt()],
                        outs=[agout[:].opt()],
                    )

                # w0's 2nd half rides the sync queue, explicitly ordered
                # BEHIND the last agin store so the tile scheduler cannot
                # hoist it ahead and delay the collective's input.
                w0b_dma = nc.sync.dma_start(out=w0_sb[:, 1, :, :], in_=w0_h[:, 1, :, :])
                add_dep_helper(w0b_dma.ins, agin_dma.ins, sync=True,
                               reason="w0b after agin stores")

                # keep the HAM clock warm across the xt/w0 DMA-wait gap
                for _ in range(10):
                    nc.tensor.matmul(
                        ps_w, lhsT=warm_sb[:, 0:128], rhs=warm_sb[:, 0:64],
                        start=True, stop=True,
                    )

                # ---------------- deep MLP (overlaps the collective) -------------
                h0_sb = persist.tile([128, 8, D1], BF16)
                ps_o = psum.tile([1, BS], F32, tag="ps_o", bufs=1)
                for mj in range(8):
                    ps0 = psum.tile([128, BS], F32, tag="ps_mm", bufs=3)
                    for t in range(KT0):
                        nc.tensor.matmul(
                            ps0,
                            lhsT=w0_sb[:, mj // 4, t, (mj % 4) * 128 : (mj % 4 + 1) * 128],
                            rhs=xt_sb[:, t, :],
                            start=(t == 0),
                            stop=(t == KT0 - 1),
                        )
                    nc.scalar.activation(
                        h0_sb[:, mj, :],
                        ps0,
                        mybir.ActivationFunctionType.Relu,
                        bias=b0_sb[:, mj : mj + 1],
                    )
                h1_sb = persist.tile([128, 4, BS], BF16)
                for mj in range(4):
                    ps1 = psum.tile([128, BS], F32, tag="ps_mm", bufs=3)
                    for t in range(8):
                        nc.tensor.matmul(
                            ps1,
                            lhsT=w1_sb[:, t, mj * 128 : (mj + 1) * 128],
                            rhs=h0_sb[:, t, :],
                            start=(t == 0),
                            stop=(t == 7),
                        )
                    nc.scalar.activation(
                        h1_sb[:, mj, :],
                        ps1,
                        mybir.ActivationFunctionType.Relu,
                        bias=b1_sb[:, mj : mj + 1],
                    )
                h2_sb = persist.tile([128, 2, BS], BF16)
                for mj in range(2):
                    ps2 = psum.tile([128, BS], F32, tag="ps_mm", bufs=3)
                    for t in range(4):
                        nc.tensor.matmul(
                            ps2,
                            lhsT=w2_sb[:, t, mj * 128 : (mj + 1) * 128],
                            rhs=h1_sb[:, t, :],
                            start=(t == 0),
                            stop=(t == 3),
                        )
                    nc.scalar.activation(
                        h2_sb[:, mj, :],
                        ps2,
                        mybir.ActivationFunctionType.Relu,
                        bias=b2_sb[:, mj : mj + 1],
                    )
                # ps_o accumulation group: deep head + linear + interaction
                for t in range(2):
                    nc.tensor.matmul(
                        ps_o,
                        lhsT=ow_sb[:, t, :],
                        rhs=h2_sb[:, t, :],
                        start=(t == 0),
                        stop=False,
                    )
                for t in range(KT0):
                    nc.tensor.matmul(
                        ps_o,
                        lhsT=wl_sb[:, t, :],
                        rhs=xt_sb[:, t, :],
                        start=False,
                        stop=False,
                    )

                # keep the HAM clock warm across the AllGather-wait gap so
                # the interaction matmuls run at full rate
                for _ in range(24):
                    nc.tensor.matmul(
                        ps_w, lhsT=warm_sb[:, 0:128], rhs=warm_sb[:, 0:64],
                        start=True, stop=True,
                    )

                # ---------------- rebuild full Sz from the AllGather -------------
                s_sb = persist.tile([P_F, KT0, F], SDT)
                engs = [nc.sync, nc.scalar]
                for i, (src, t, po, n) in enumerate(reload_segs):
                    engs[i % len(engs)].dma_start(
                        out=s_sb[po : po + n, t, :], in_=agout[src : src + n, :],
                        single_packet=True,
                    )
                for t in range(KT0):
                    nc.gpsimd.affine_select(
                        out=s_sb[:, t, t * P_F : (t + 1) * P_F],
                        in_=s_sb[:, t, t * P_F : (t + 1) * P_F],
                        compare_op=mybir.AluOpType.not_equal,
                        fill=0.0,
                        base=0,
                        pattern=[[-1, P_F]],
                        channel_multiplier=1,
                    )

                # ---------------- interaction: YT = Sz @ XT, 0.5*colsum(YT*XT) ---
                for mj in range(KT0):
                    ps_y = psum.tile([P_F, BS], F32, tag="ps_mm", bufs=3)
                    for t in range(KT0):
                        nc.tensor.matmul(
                            ps_y,
                            lhsT=s_sb[:, t, mj * P_F : (mj + 1) * P_F],
                            rhs=xt_sb[:, t, :],
                            start=(t == 0),
                            stop=(t == KT0 - 1),
                        )
                    z_sb = work.tile([P_F, BS], XDT, tag="z")
                    nc.vector.tensor_mul(z_sb, ps_y, xt_sb[:, mj, :])
                    nc.tensor.matmul(
                        ps_o,
                        lhsT=halves,
                        rhs=z_sb,
                        start=False,
                        stop=(mj == KT0 - 1),
                    )

                # ---------------- final: add folded scalar bias, store -----------
                out_sb = persist.tile([1, BS], F32)
                nc.vector.tensor_scalar_add(out_sb, ps_o, float(bias_const))
                nc.sync.dma_start(out=out_h[:], in_=out_sb)

    nc.compile()
    return nc


def kernel(X, w1, b, nfk, f2f, deepW0, deepB0, deepW1, deepB1, deepW2, deepB2,
           outW, outB, **_unused):
    import ml_dtypes
    bf16 = ml_dtypes.bfloat16

    X = np.ascontiguousarray(X, dtype=np.float32)
    w1 = np.asarray(w1, dtype=np.float32)
    b = np.asarray(b, dtype=np.float32)
    nfk = np.ascontiguousarray(nfk, dtype=np.float32)
    f2f = np.asarray(f2f)
    deepW0 = np.ascontiguousarray(deepW0, dtype=np.float32)
    deepW1 = np.ascontiguousarray(deepW1, dtype=np.float32)
    deepW2 = np.ascontiguousarray(deepW2, dtype=np.float32)
    outW = np.ascontiguousarray(outW, dtype=np.float32)

    # ---- host-side layout transforms (index/permutation/cast work only) ----
    perm = np.argsort(f2f, kind="stable")
    counts = np.bincount(np.asarray(f2f, dtype=np.int64), minlength=NGT)[:NGT]
    off = np.zeros(NGT + 1, dtype=np.int64)
    off[1:] = np.cumsum(counts)
    pad = int(max(counts.max(), 1))
    lrows = GPC * pad

    XT = np.ascontiguousarray(X[:, perm].T)                     # [F, B]
    w1p = np.ascontiguousarray(w1[perm].reshape(F, 1))
    nfkp = nfk[perm]                                            # [F, FIELDS, K]
    nfkT = np.ascontiguousarray(nfkp.reshape(F, FIELDS * K).T)  # [FIELDS*K, F]
    W0p = np.ascontiguousarray(deepW0[perm])
    bias_const = float(np.float32(b[0]) + np.float32(outB[0]))

    nc = _build_program(off, counts, pad, bias_const)

    def _c(a, dt=bf16):
        return np.ascontiguousarray(a).astype(dt)

    w0_dev = _c(W0p.reshape(KT0, P_F, 2, D1).transpose(1, 2, 0, 3))
    w1_dev = _c(deepW1.reshape(8, 128, D1).transpose(1, 0, 2))
    w2_dev = _c(deepW2.reshape(4, 128, D2).transpose(1, 0, 2))
    ow_dev = _c(outW.reshape(2, 128, 1).transpose(1, 0, 2))
    wl_dev = _c(w1p.reshape(KT0, P_F, 1).transpose(1, 0, 2))
    b0_dev = np.ascontiguousarray(np.asarray(deepB0, np.float32).reshape(8, 128).T)
    b1_dev = np.ascontiguousarray(np.asarray(deepB1, np.float32).reshape(4, 128).T)
    b2_dev = np.ascontiguousarray(np.asarray(deepB2, np.float32).reshape(2, 128).T)
    halves_dev = np.full((P_F, 1), 0.5, dtype=bf16)

    in_maps = []
    for c in range(NCORES):
        nk_cols = np.zeros((FIELDS * K, lrows), dtype=np.float32)
        gtiles = np.zeros((GPC * K, F), dtype=np.float32)
        for gl in range(GPC):
            g = c * GPC + gl
            if g >= FIELDS or counts[g] == 0:
                continue
            nk_cols[:, gl * pad : gl * pad + counts[g]] = (
                nfkT[:, off[g] : off[g + 1]]
            )
            gtiles[gl * K : (gl + 1) * K, :] = nfkT[g * K : (g + 1) * K, :]
        in_maps.append({
            "xt": _c(XT[:, c * BS : (c + 1) * BS].reshape(KT0, P_F, BS).transpose(1, 0, 2)),
            "w0": w0_dev,
            "w1w": w1_dev,
            "w2w": w2_dev,
            "outw": ow_dev,
            "w1lin": wl_dev,
            "b0": b0_dev, "b1": b1_dev, "b2": b2_dev,
            "nfkt_cols": _c(nk_cols.reshape(FIELDS, K, lrows).transpose(1, 0, 2)),
            "gtiles": _c(gtiles.reshape(GPC, K, F).transpose(1, 0, 2)),
            "halves": halves_dev,
        })

    res = run_bass_kernel_spmd(nc, in_maps, core_ids=list(range(NCORES)))
    global LAST_RESULT
    LAST_RESULT = res
    out = np.concatenate([r["out"].reshape(-1) for r in res.results])
    return out.astype(np.float32)


LAST_RESULT = None


if __name__ == "__main__":
    import importlib.util as _iu

    spec = _iu.spec_from_file_location("ref", "/root/problem/reference.py")
    ref = _iu.module_from_spec(spec)
    spec.loader.exec_module(ref)
    inp = {k: np.asarray(v) for k, v in ref.setup_inputs().items()}
    got = kernel(**inp)
    print("kernel out:", got[:8])
